# revision 9
# baseline (speedup 1.0000x reference)
"""DLRM pairwise-interaction layer on 8 Trainium2 NeuronCores.

Computes, for each batch row b, the strict upper triangle of the Gram matrix
G_b = E_b @ E_b.T where E_b is (27 features, 128 dims), i.e. the reference

    interactions = einsum("bfd,bgd->bfg", E, E);  out = interactions[:, triu_i, triu_j]

Strategy (pure batch data-parallel, 2048 rows/core), bf16, UNPADDED stream:
  * Host transposes to (128, rows*27) bf16 -- no feature padding.  Total
    input DMA is 14.16 MB/core (vs 16.8 MB padded), the kernel's pacer.
  * 4 batch rows per matmul group: stationary = a contiguous 128-col window
    at stride 108 (the group's 108 cols + 20-col overlap into the next
    group; NumWeights==128 keeps fast-weight-load); moving = the group's
    108 cols (N=108).  The four wanted 27x27 diagonal blocks land at PSUM
    (parts 27q, cols 27q); junk partitions 108..127 are never read.
  * Engine PSUM reads must start at a 32-aligned partition; spans starting
    at partition 0 may be any size, and engine cost depends only on the
    free-dim size (partitions are parallel lanes).  So every extraction
    copy starts at partition 0 with span {27,54,81,108} per q.
  * Per 64-row pass-tile (16 matmuls, 4 PSUM banks): VectorE extracts banks
    0-1 and ScalarE banks 2-3 concurrently (different banks -> legal), 4
    q-copies each, col slices 27q+1..27q+26 (g=0 dropped; only f<g needed).
    psum bufs=2 lets PE fill banks 4-7 meanwhile.
  * Out tile is laid out q-outermost so the per-q output DMA (slicing the
    27 good partitions) is one contiguous multi-KB run per partition.
    Output is 2.8 MB/core.
  * Chunk sizes ramp 64..256..64 so the first matmul starts after ~0.4 MB
    of DMA instead of 2 MB, and the tail drains quickly.
"""

import numpy as np

B = 16384
F = 27
GO = 26                      # g columns kept per block (g = 1..26)
D = 128
NCORES = 8
BLOC = B // NCORES           # 2048 batch rows per core
CHUNK_ROWS = [64, 64, 128] + [256] * 6 + [128, 64, 64]   # sums to 2048
assert sum(CHUNK_ROWS) == BLOC
NTILE = BLOC // 64           # 32 pass-tiles of 64 rows
ET_COLS = BLOC * F + 128     # unpadded stream + tail pad for last window

QSPAN = [27, 54, 81, 108]    # psum partition span per q (start always 0)

_TRIU_I, _TRIU_J = np.triu_indices(F, k=1)

_compiled = None


def _build():
    import concourse.bacc as bacc
    import concourse.mybir as mybir
    from concourse.tile import TileContext

    f32 = mybir.dt.float32
    bf16 = mybir.dt.bfloat16
    nc = bacc.Bacc(None, target_bir_lowering=False)

    et = nc.dram_tensor("et", [D, ET_COLS], bf16, kind="ExternalInput")
    y = nc.dram_tensor("y", [4, F, NTILE, 4, 4, GO], bf16,
                       kind="ExternalOutput")

    from concourse.ap import AP

    with TileContext(nc) as tc:
        with (
            tc.tile_pool(name="inp", bufs=8) as inp,
            tc.tile_pool(name="outp", bufs=3) as outp,
            tc.tile_pool(name="psum", bufs=1, space="PSUM") as psum,
        ):
            # one persistent 8-bank PSUM tile; banks indexed explicitly so
            # dependency tracking is per-bank (slice-level), letting the PE
            # refill a bank as soon as its extraction copy completes.
            ps = psum.tile([D, 8, 4, 128], f32)
            PP = 8 * 4 * 128            # psum per-partition pitch (elems)
            OP = 4 * 4 * 4 * 4 * GO     # out_t per-partition pitch (elems)
            TCOLS = 64 * F              # 1728 cols per pass-tile
            CSTART = [sum(CHUNK_ROWS[:i]) for i in range(len(CHUNK_ROWS))]

            in_tiles = {}

            def emit_in(ci):
                rows_c = CHUNK_ROWS[ci]
                in_t = inp.tile([D, 256 * F + 20], bf16)
                c0 = CSTART[ci] * F
                nc.sync.dma_start(
                    in_t[:, :rows_c * F + 20],
                    et[:, c0:c0 + rows_c * F + 20],
                )
                in_tiles[ci] = in_t

            # prefetch 4 chunks; inside iteration c, chunk c+4's input DMA is
            # emitted BEFORE chunk c's output DMAs so the Sync ring's FIFO
            # never stalls input prefetch behind an output's sem-wait.
            for ci in range(4):
                emit_in(ci)
            for c, rows_c in enumerate(CHUNK_ROWS):
                if c + 4 < len(CHUNK_ROWS):
                    emit_in(c + 4)
                npass = rows_c // 64
                in_t = in_tiles.pop(c)
                # part, q, pass, bank, slot, g  (q outer: per-q DMA contiguous)
                out_t = outp.tile([D, 4, 4, 4, 4, GO], bf16)
                for hh in range(npass):
                    tctr = CSTART[c] // 64 + hh
                    b0 = 4 * (tctr % 2)          # bank group for this tile
                    for m in range(16):
                        g = 16 * hh + m
                        stat = in_t[:, 108 * g:108 * g + 128]
                        mov = in_t[:, 108 * g:108 * g + 108]
                        nc.tensor.matmul(ps[:, b0 + m // 4, m % 4, 0:108],
                                         stat, mov, start=True, stop=True)
                    # one merged copy per engine per tile: dims
                    # [part, bankslot(8), q(4), g(26)] -- bank stride 512 =
                    # 4*slot stride and out bk stride 104 = 4*s stride, so
                    # (bank, slot) merge into one AP dim.  V reads banks
                    # b0..b0+1 while S reads b0+2..b0+3 (different banks),
                    # and each half-tile frees for the PE independently.
                    for half in range(2):
                        sb = ps[0:108, b0 + 2 * half, 0, :]
                        csrc = AP(tensor=sb.tensor, offset=sb.offset + 1,
                                  ap=[[PP, 108], [128, 8], [27, 4], [1, GO]])
                        db = out_t[0:108, 0, hh, 2 * half, 0, :]
                        cdst = AP(tensor=db.tensor, offset=db.offset,
                                  ap=[[OP, 108], [GO, 8],
                                      [4 * 4 * 4 * GO, 4], [1, GO]])
                        if half == 0:
                            nc.vector.tensor_copy(cdst, csrc)
                        else:
                            nc.scalar.copy(cdst, csrc)
                # per-q output DMAs (clean 27-partition slices) on the Sync
                # ring, behind the next chunks' input DMAs
                t0 = CSTART[c] // 64
                for q in range(4):
                    nc.sync.dma_start(
                        y[q, :, t0:t0 + npass, :, :, :],
                        out_t[27 * q:27 * q + F, q, :npass, :, :, :],
                    )

    nc.compile()
    return nc


def _get_compiled():
    global _compiled
    if _compiled is None:
        _compiled = _build()
    return _compiled


def _prep_inputs(embeddings: np.ndarray):
    """Full (B, 27, 128) fp32 -> per-core unpadded bf16 (128, ET_COLS)."""
    import ml_dtypes

    bf16 = ml_dtypes.bfloat16
    e = np.asarray(embeddings, dtype=np.float32)
    # (D, B, F) bf16
    eT = np.ascontiguousarray(e.transpose(2, 0, 1)).astype(bf16)
    in_maps = []
    for c in range(NCORES):
        etc = np.zeros((D, ET_COLS), dtype=bf16)
        etc[:, :BLOC * F] = eT[:, c * BLOC:(c + 1) * BLOC, :].reshape(
            D, BLOC * F
        )
        in_maps.append({"et": etc})
    return in_maps


def _decode_core(yv: np.ndarray) -> np.ndarray:
    """(4, 27, NTILE, 4, 4, GO) bf16 -> (BLOC, 351) fp32."""
    g = np.asarray(yv).astype(np.float32)
    # row = 64*t + 16*bk + 4*s + q ; g[q, f, t, bk, s, j] = G[row, f, j+1]
    g = g.transpose(2, 3, 4, 0, 1, 5).reshape(BLOC, F, GO)
    return g[:, _TRIU_I, _TRIU_J - 1]


def kernel(embeddings: np.ndarray) -> np.ndarray:
    from concourse.bass_utils import run_bass_kernel_spmd

    nc = _get_compiled()
    in_maps = _prep_inputs(embeddings)
    res = run_bass_kernel_spmd(nc, in_maps, core_ids=list(range(NCORES)))

    out = np.empty((B, len(_TRIU_I)), dtype=np.float32)
    for c in range(NCORES):
        out[c * BLOC:(c + 1) * BLOC] = _decode_core(res.results[c]["y"])
    return out


# revision 10
# speedup vs baseline: 1.0899x; 1.0899x over previous
"""DLRM pairwise-interaction layer on 8 Trainium2 NeuronCores.

Computes, for each batch row b, the strict upper triangle of the Gram matrix
G_b = E_b @ E_b.T where E_b is (27 features, 128 dims), i.e. the reference

    interactions = einsum("bfd,bgd->bfg", E, E);  out = interactions[:, triu_i, triu_j]

Strategy (pure batch data-parallel, 2048 rows/core), bf16, UNPADDED stream:
  * Host transposes to (128, rows*27) bf16 -- no feature padding.  Total
    input DMA is 14.16 MB/core (vs 16.8 MB padded), the kernel's pacer.
  * 4 batch rows per matmul group: stationary = a contiguous 128-col window
    at stride 108 (the group's 108 cols + 20-col overlap into the next
    group; NumWeights==128 keeps fast-weight-load); moving = the group's
    108 cols (N=108).  The four wanted 27x27 diagonal blocks land at PSUM
    (parts 27q, cols 27q); junk partitions 108..127 are never read.
  * Engine PSUM reads must start at a 32-aligned partition; spans starting
    at partition 0 may be any size, and engine cost depends only on the
    free-dim size (partitions are parallel lanes).  So every extraction
    copy starts at partition 0 with span {27,54,81,108} per q.
  * Per 64-row pass-tile (16 matmuls, 4 PSUM banks): VectorE extracts banks
    0-1 and ScalarE banks 2-3 concurrently (different banks -> legal), 4
    q-copies each, col slices 27q+1..27q+26 (g=0 dropped; only f<g needed).
    psum bufs=2 lets PE fill banks 4-7 meanwhile.
  * Out tile is laid out q-outermost so the per-q output DMA (slicing the
    27 good partitions) is one contiguous multi-KB run per partition.
    Output is 2.8 MB/core.
  * Chunk sizes ramp 64..256..64 so the first matmul starts after ~0.4 MB
    of DMA instead of 2 MB, and the tail drains quickly.
"""

import numpy as np

B = 16384
F = 27
GO = 26                      # g columns kept per block (g = 1..26)
D = 128
NCORES = 8
BLOC = B // NCORES           # 2048 batch rows per core
CHUNK_ROWS = [64, 64, 128] + [256] * 6 + [128, 64, 64]   # sums to 2048
assert sum(CHUNK_ROWS) == BLOC
NTILE = BLOC // 64           # 32 pass-tiles of 64 rows
ET_COLS = BLOC * F + 128     # unpadded stream + tail pad for last window

QSPAN = [27, 54, 81, 108]    # psum partition span per q (start always 0)

_TRIU_I, _TRIU_J = np.triu_indices(F, k=1)

_compiled = None


def _build():
    import concourse.bacc as bacc
    import concourse.mybir as mybir
    from concourse.tile import TileContext

    f32 = mybir.dt.float32
    bf16 = mybir.dt.bfloat16
    nc = bacc.Bacc(None, target_bir_lowering=False)

    et = nc.dram_tensor("et", [D, ET_COLS], bf16, kind="ExternalInput")
    y = nc.dram_tensor("y", [4, F, NTILE, 4, 4, GO], bf16,
                       kind="ExternalOutput")

    from concourse.ap import AP

    with TileContext(nc) as tc:
        with (
            tc.tile_pool(name="inp", bufs=8) as inp,
            tc.tile_pool(name="outp", bufs=2) as outp,
            tc.tile_pool(name="psum", bufs=1, space="PSUM") as psum,
        ):
            # one persistent 8-bank PSUM tile; banks indexed explicitly so
            # dependency tracking is per-bank (slice-level), letting the PE
            # refill a bank as soon as its extraction copy completes.
            ps = psum.tile([D, 8, 4, 128], f32)
            PP = 8 * 4 * 128            # psum per-partition pitch (elems)
            OP = 4 * 8 * 4 * 4 * GO     # out_t per-partition pitch (elems)
            TCOLS = 64 * F              # 1728 cols per pass-tile
            CSTART = [sum(CHUNK_ROWS[:i]) for i in range(len(CHUNK_ROWS))]

            in_tiles = {}

            def emit_in(ci):
                rows_c = CHUNK_ROWS[ci]
                in_t = inp.tile([D, 256 * F + 20], bf16)
                c0 = CSTART[ci] * F
                nc.sync.dma_start(
                    in_t[:, :rows_c * F + 20],
                    et[:, c0:c0 + rows_c * F + 20],
                )
                in_tiles[ci] = in_t

            # prefetch 4 chunks; inside iteration c, chunk c+4's input DMA is
            # emitted BEFORE chunk c's output DMAs so the Sync ring's FIFO
            # never stalls input prefetch behind an output's sem-wait.
            for ci in range(4):
                emit_in(ci)
            for c, rows_c in enumerate(CHUNK_ROWS):
                if c + 4 < len(CHUNK_ROWS):
                    emit_in(c + 4)
                npass = rows_c // 64
                in_t = in_tiles.pop(c)
                for hh in range(npass):
                    tctr = CSTART[c] // 64 + hh
                    if tctr % 8 == 0:
                        # part, q, tile-in-group(8), bank, slot, g
                        out_t = outp.tile([D, 4, 8, 4, 4, GO], bf16)
                    b0 = 4 * (tctr % 2)          # bank group for this tile
                    for m in range(16):
                        g = 16 * hh + m
                        stat = in_t[:, 108 * g:108 * g + 128]
                        mov = in_t[:, 108 * g:108 * g + 108]
                        nc.tensor.matmul(ps[:, b0 + m // 4, m % 4, 0:108],
                                         stat, mov, start=True, stop=True)
                    # one merged copy per engine per tile: dims
                    # [part, bankslot(8), q(4), g(26)] -- bank stride 512 =
                    # 4*slot stride and out bk stride 104 = 4*s stride, so
                    # (bank, slot) merge into one AP dim.  V reads banks
                    # b0..b0+1 while S reads b0+2..b0+3 (different banks),
                    # and each half-tile frees for the PE independently.
                    for half in range(2):
                        sb = ps[0:108, b0 + 2 * half, 0, :]
                        csrc = AP(tensor=sb.tensor, offset=sb.offset + 1,
                                  ap=[[PP, 108], [128, 8], [27, 4], [1, GO]])
                        db = out_t[0:108, 0, tctr % 8, 2 * half, 0, :]
                        cdst = AP(tensor=db.tensor, offset=db.offset,
                                  ap=[[OP, 108], [GO, 8],
                                      [8 * 4 * 4 * GO, 4], [1, GO]])
                        if half == 0:
                            nc.vector.tensor_copy(cdst, csrc)
                        else:
                            nc.scalar.copy(cdst, csrc)
                    # per-q output DMAs for each 8-tile group (512 rows) on
                    # the Sync ring, behind the next chunks' input DMAs
                    if tctr % 8 == 7:
                        gi = tctr // 8
                        for q in range(4):
                            nc.sync.dma_start(
                                y[q, :, 8 * gi:8 * gi + 8, :, :, :],
                                out_t[27 * q:27 * q + F, q, :, :, :, :],
                            )

    nc.compile()
    return nc


def _get_compiled():
    global _compiled
    if _compiled is None:
        _compiled = _build()
    return _compiled


def _prep_inputs(embeddings: np.ndarray):
    """Full (B, 27, 128) fp32 -> per-core unpadded bf16 (128, ET_COLS)."""
    import ml_dtypes

    bf16 = ml_dtypes.bfloat16
    e = np.asarray(embeddings, dtype=np.float32)
    # (D, B, F) bf16
    eT = np.ascontiguousarray(e.transpose(2, 0, 1)).astype(bf16)
    in_maps = []
    for c in range(NCORES):
        etc = np.zeros((D, ET_COLS), dtype=bf16)
        etc[:, :BLOC * F] = eT[:, c * BLOC:(c + 1) * BLOC, :].reshape(
            D, BLOC * F
        )
        in_maps.append({"et": etc})
    return in_maps


def _decode_core(yv: np.ndarray) -> np.ndarray:
    """(4, 27, NTILE, 4, 4, GO) bf16 -> (BLOC, 351) fp32."""
    g = np.asarray(yv).astype(np.float32)
    # row = 64*t + 16*bk + 4*s + q ; g[q, f, t, bk, s, j] = G[row, f, j+1]
    g = g.transpose(2, 3, 4, 0, 1, 5).reshape(BLOC, F, GO)
    return g[:, _TRIU_I, _TRIU_J - 1]


def kernel(embeddings: np.ndarray) -> np.ndarray:
    from concourse.bass_utils import run_bass_kernel_spmd

    nc = _get_compiled()
    in_maps = _prep_inputs(embeddings)
    res = run_bass_kernel_spmd(nc, in_maps, core_ids=list(range(NCORES)))

    out = np.empty((B, len(_TRIU_I)), dtype=np.float32)
    for c in range(NCORES):
        out[c * BLOC:(c + 1) * BLOC] = _decode_core(res.results[c]["y"])
    return out
